# revision 7
# baseline (speedup 1.0000x reference)
"""Multi-head attention (B=2, T=2048, H=1024, 16 heads) on 8 trn2 cores.

Sharding: data-parallel over batch (2) x tensor-parallel over head groups
(4 heads/core).  Each core computes the qkv projection for its 4 heads,
attention, and a partial out-projection; the host sums the 4 partials per
batch and adds the bias constants.

Key optimizations over the straightforward layout:
  * The mask is per-key and known on the host: tokens are permuted so
    valid keys come first, and k/v projections + scores + exp + AV run
    only over ceil(nv/128)*128 keys (~1152 of 2048).  Exact: masked keys
    contribute exp(-1e9)=0.  Queries/outputs use all tokens (computed in
    permuted order, unpermuted on the host).
  * x is transposed on the host (xT [H, T]) -> no PE transposes at all.
  * v is projected directly in key-major layout [tok, feat] using xT
    blocks as the stationary operand -> no V transposes; a ones column
    per key tile gives the softmax denominator for free in the AV matmul.
  * x and W_qkv are bf16 (same PE rate as f32r, half the DMA/SBUF);
    scores/AV/out-projection stay f32r.
  * b_k is dropped (softmax shift invariance - exact), b_v is folded into
    a host-side output constant (softmax weights sum to 1 - exact), b_q
    is applied via the activation bias on the qT copy.
  * PE emission interleaves projection / out-projection chunks between
    each score block and its AV block so the PE never stalls on the
    scalar-engine exp pipeline.
"""

import sys

sys.path.insert(0, "/opt/trn_rl_repo")

import numpy as np

B, T, H = 2, 2048, 1024
NH, DK = 16, 64
HPC = 4            # heads per core
NCORES = 8
NT = T // 128      # 16 token tiles
KTH = H // 128     # 8 hidden contraction tiles

_CACHE = {}


def _build(nkt):
    """Build the per-core kernel for nkt key tiles (KP = nkt*128 keys)."""
    import concourse.bacc as bacc
    import concourse.mybir as mybir
    import concourse.tile as tile

    f32 = mybir.dt.float32
    f32r = mybir.dt.float32r
    bf16 = mybir.dt.bfloat16
    AF = mybir.ActivationFunctionType
    ALU = mybir.AluOpType

    KP = nkt * 128

    nc = bacc.Bacc("TRN2", target_bir_lowering=False, debug=False)

    xt_d = nc.dram_tensor("xt", [H, T], bf16, kind="ExternalInput")
    wq_d = nc.dram_tensor("wq", [H, 256], bf16, kind="ExternalInput")
    wk_d = nc.dram_tensor("wk", [H, 256], bf16, kind="ExternalInput")
    wv_d = nc.dram_tensor("wv", [H, 256], bf16, kind="ExternalInput")
    wout_d = nc.dram_tensor("wout", [256, H], f32r, kind="ExternalInput")
    bq_d = nc.dram_tensor("bqt", [128, 2], f32, kind="ExternalInput")
    maskb_d = nc.dram_tensor("maskbias", [128, nkt], f32, kind="ExternalInput")
    out_d = nc.dram_tensor("out_partial", [T, H], f32, kind="ExternalOutput")

    # k-projection column chunks (cover KP, each >=256 wide for f32r rate)
    if nkt % 3 == 0:
        kchunks = [(i * KP // 3, KP // 3) for i in range(3)]
    else:
        kchunks = [(o, min(512, KP - o)) for o in range(0, KP, 512)]

    with tile.TileContext(nc) as tc:
        with (
            tc.tile_pool(name="persist", bufs=1) as pp,
            tc.tile_pool(name="small", bufs=1) as sp,
            tc.tile_pool(name="expp", bufs=10) as ep,
            tc.tile_pool(name="recipp", bufs=2) as rp,
            tc.tile_pool(name="ostage", bufs=4) as osp,
            tc.tile_pool(name="psum", bufs=1, space="PSUM") as psp,
        ):
            ones_f = sp.tile([128, 64], f32, tag="ones_f", name="ones_f")
            nc.vector.memset(ones_f, 1.0)
            ones = sp.tile([1, 64], f32r, tag="ones", name="ones")
            nc.scalar.copy(ones, ones_f[0:1, :])
            bqt = sp.tile([128, 2], f32, tag="bqt", name="bqt")
            nc.sync.dma_start(out=bqt, in_=bq_d[:, :])
            maskb = sp.tile([128, nkt], f32, tag="maskb", name="maskb")
            nc.sync.dma_start(out=maskb, in_=maskb_d[:, :])

            # weights (gpsimd queue): small first, wout last
            wq = sp.tile([128, KTH * 256], bf16, tag="wq", name="wq")
            wk = sp.tile([128, KTH * 256], bf16, tag="wk", name="wk")
            wv = sp.tile([128, KTH * 256], bf16, tag="wv", name="wv")
            for w_s, w_d in ((wk, wk_d), (wv, wv_d), (wq, wq_d)):
                for kt in range(KTH):
                    nc.gpsimd.dma_start(
                        out=w_s[:, kt * 256 : (kt + 1) * 256],
                        in_=w_d[kt * 128 : (kt + 1) * 128, :],
                    )
            wout = sp.tile([128, 2 * H], f32r, tag="wout", name="wout")
            for p in range(2):
                nc.gpsimd.dma_start(
                    out=wout[:, p * H : (p + 1) * H],
                    in_=wout_d[p * 128 : (p + 1) * 128, :],
                )

            # xT (sync queue), chunk-major so k/v/q chunk c of every hidden
            # tile lands before chunk c+1 of any.
            xT = pp.tile([128, KTH * T], bf16, tag="xT", name="xT")
            xcols = [(o, w) for (o, w) in kchunks]
            o_end = kchunks[-1][0] + kchunks[-1][1]
            while o_end < T:
                w = min(512, T - o_end)
                xcols.append((o_end, w))
                o_end += w
            for o, w in xcols:
                for kt in range(KTH):
                    nc.sync.dma_start(
                        out=xT[:, kt * T + o : kt * T + o + w],
                        in_=xt_d[kt * 128 : (kt + 1) * 128, o : o + w],
                    )

            # persistent activations
            qT = [
                pp.tile([128, T], f32r, tag=f"qT{m}", name=f"qT{m}")
                for m in range(2)
            ]
            kT = [
                pp.tile([128, KP], f32r, tag=f"kT{m}", name=f"kT{m}")
                for m in range(2)
            ]
            vp = [
                pp.tile([128, nkt * 65], f32r, tag=f"vp{h}", name=f"vp{h}")
                for h in range(HPC)
            ]
            for h in range(HPC):
                # ones column at slot 64 of each 65-wide key tile block
                vpv = vp[h].rearrange("p (t c) -> p t c", c=65)
                nc.scalar.copy(vpv[:, :, 64], ones_f[:, 0:nkt])
            attn = [
                pp.tile([128, T], f32r, tag=f"attnp{p}", name=f"attnp{p}")
                for p in range(2)
            ]

            def qproj_chunk(mt, c):
                # qT[mt][:, c*512:+512]
                ps = psp.tile([128, 512], f32, tag="ps", bufs=2, name="ps")
                for kt in range(KTH):
                    nc.tensor.matmul(
                        ps,
                        wq[:, kt * 256 + mt * 128 : kt * 256 + (mt + 1) * 128],
                        xT[:, kt * T + c * 512 : kt * T + c * 512 + 512],
                        start=(kt == 0),
                        stop=(kt == KTH - 1),
                    )
                nc.scalar.activation(
                    qT[mt][:, c * 512 : c * 512 + 512],
                    ps,
                    AF.Identity,
                    bias=bqt[:, mt : mt + 1],
                    scale=1.0,
                )

            def kproj_chunk(mt, c):
                o, w = kchunks[c]
                ps = psp.tile([128, 512], f32, tag="ps", bufs=2, name="ps")
                for kt in range(KTH):
                    nc.tensor.matmul(
                        ps[:, 0:w],
                        wk[:, kt * 256 + mt * 128 : kt * 256 + (mt + 1) * 128],
                        xT[:, kt * T + o : kt * T + o + w],
                        start=(kt == 0),
                        stop=(kt == KTH - 1),
                    )
                nc.vector.tensor_copy(kT[mt][:, o : o + w], ps[:, 0:w])

            def vproj_tile(tt):
                # v for all 4 heads at token tile tt, direct [tok, feat]
                pv = psp.tile([128, 512], f32, tag="ps", bufs=2, name="pv")
                for kt in range(KTH):
                    nc.tensor.matmul(
                        pv[:, 0:256],
                        xT[:, kt * T + tt * 128 : kt * T + (tt + 1) * 128],
                        wv[:, kt * 256 : (kt + 1) * 256],
                        start=(kt == 0),
                        stop=(kt == KTH - 1),
                    )
                for h in range(HPC):
                    nc.vector.tensor_copy(
                        vp[h][:, tt * 65 : tt * 65 + 64],
                        pv[:, h * 64 : (h + 1) * 64],
                    )

            def scores_block(hp, nb):
                # all key tiles of one 512-query block, both heads of pair
                exs = []
                for kt in range(nkt):
                    ss = psp.tile([128, 1024], f32, tag="ss", bufs=2, name="ss")
                    for lh in range(2):
                        r0 = lh * 64
                        nc.tensor.matmul(
                            ss[:, lh * 512 : (lh + 1) * 512],
                            kT[hp][r0 : r0 + 64, kt * 128 : (kt + 1) * 128],
                            qT[hp][r0 : r0 + 64, nb * 512 : nb * 512 + 512],
                            start=True,
                            stop=True,
                        )
                    ex = ep.tile([128, 1024], f32r, tag="ex", name="ex")
                    nc.scalar.activation(
                        ex, ss, AF.Exp, bias=maskb[:, kt : kt + 1], scale=1.0
                    )
                    exs.append(ex)
                return exs

            def av_block(hp, nb, exs):
                acc = psp.tile([65, 1024], f32, tag="acc", bufs=1, name="acc")
                for kt in range(nkt):
                    for lh in range(2):
                        h = hp * 2 + lh
                        nc.tensor.matmul(
                            acc[:, lh * 512 : (lh + 1) * 512],
                            vp[h][:, kt * 65 : kt * 65 + 65],
                            exs[kt][:, lh * 512 : (lh + 1) * 512],
                            start=(kt == 0),
                            stop=(kt == nkt - 1),
                        )
                for lh in range(2):
                    r0 = lh * 64
                    rec = rp.tile([1, 512], f32r, tag="rec", name="rec")
                    with nc.allow_low_precision(
                        reason="f32r rounding for PE broadcast"
                    ):
                        nc.vector.reciprocal(
                            rec, acc[64:65, lh * 512 : lh * 512 + 512]
                        )
                    pb = psp.tile([64, 512], f32, tag="ps", bufs=2, name="pb")
                    nc.tensor.matmul(pb, ones, rec, start=True, stop=True)
                    recb = rp.tile([64, 512], f32, tag="recb", name="recb")
                    nc.vector.tensor_copy(recb, pb)
                    nc.vector.tensor_tensor(
                        out=attn[hp][r0 : r0 + 64, nb * 512 : nb * 512 + 512],
                        in0=acc[0:64, lh * 512 : lh * 512 + 512],
                        in1=recb,
                        op=ALU.mult,
                    )

            def outproj_mt(mt):
                for ob in range(2):
                    po = psp.tile([128, 512], f32, tag="ps", bufs=2, name="po")
                    for p in range(2):
                        nc.tensor.matmul(
                            po,
                            attn[p][:, mt * 128 : (mt + 1) * 128],
                            wout[:, p * H + ob * 512 : p * H + ob * 512 + 512],
                            start=(p == 0),
                            stop=(p == 1),
                        )
                    ot = osp.tile([128, 512], f32, tag="ot", name="ot")
                    nc.any.tensor_copy(ot, po)
                    nc.sync.dma_start(
                        out=out_d[
                            mt * 128 : (mt + 1) * 128, ob * 512 : ob * 512 + 512
                        ],
                        in_=ot,
                    )

            # ---- schedule ----
            # Emission order defines dependencies: everything a block
            # reads must be emitted before it.  Filler units (PE work
            # whose consumers come later) are emitted between each score
            # block and its AV block so the PE stays busy while the
            # scalar engine runs the exp pipeline.

            # Phase A: the minimum attention pair 0 block 0 needs.
            for c in range(len(kchunks)):
                kproj_chunk(0, c)
            vproj_tile(0)
            vproj_tile(1)
            qproj_chunk(0, 0)

            # filler whose consumers are all in attention pair 1
            filler = [
                lambda c=c: kproj_chunk(1, c) for c in range(len(kchunks))
            ] + [lambda c=c: qproj_chunk(1, c) for c in range(4)]
            fi = [0]

            def take_filler(n):
                while n > 0 and fi[0] < len(filler):
                    filler[fi[0]]()
                    fi[0] += 1
                    n -= 1

            # Phase B: attention pair 0, pair-1 projections as filler.
            for nb in range(4):
                if nb > 0:
                    qproj_chunk(0, nb)  # consumed by this score block
                exs = scores_block(0, nb)
                if nb == 0:
                    # remaining v tiles: consumed by this AV block
                    for tt in range(2, nkt):
                        vproj_tile(tt)
                else:
                    take_filler(3 if nb < 3 else 2)
                av_block(0, nb, exs)

            # Phase C: attention pair 1, out-projection as filler.
            for nb in range(4):
                exs = scores_block(1, nb)
                take_filler(len(filler))  # leftovers, if any
                if nb > 0:
                    for mt in range((nb - 1) * 4, nb * 4):
                        outproj_mt(mt)
                av_block(1, nb, exs)

            # Phase D: tail out-projection.
            for mt in range(12, 16):
                outproj_mt(mt)

    nc.compile()
    return nc


def _get_nc(nkt):
    key = f"nc{nkt}"
    if key not in _CACHE:
        _CACHE[key] = _build(nkt)
    return _CACHE[key]


def _plan(mask):
    """Per-batch token permutation (valid keys first) and key-tile count."""
    m = np.asarray(mask)[:, 0, 0, :]
    perms, nvs = [], []
    for b in range(B):
        valid = np.nonzero(m[b] != 0)[0]
        invalid = np.nonzero(m[b] == 0)[0]
        perms.append(np.concatenate([valid, invalid]))
        nvs.append(len(valid))
    nkt = max(1, -(-max(nvs) // 128))  # ceil; same program on all cores
    nkt = min(nkt, NT)
    return perms, nvs, nkt


def _prep_in_maps(x, mask, W_qkv, b_qkv, W_out, perms, nvs, nkt):
    import ml_dtypes

    bf16 = ml_dtypes.bfloat16
    KP = nkt * 128
    xts = []
    mbs = []
    for b in range(B):
        xts.append(np.ascontiguousarray(x[b][perms[b]].T.astype(bf16)))
        mb = np.zeros(KP, dtype=np.float32)
        mb[nvs[b] :] = -1e9
        mbs.append(np.ascontiguousarray(mb.reshape(nkt, 128).T))

    in_maps = []
    for c in range(NCORES):
        b = c // 4
        h0 = (c % 4) * HPC
        cols = np.arange(h0 * DK, (h0 + HPC) * DK)
        wq = W_qkv[:, cols].astype(bf16)
        wk = (W_qkv[:, H + cols] * 0.125).astype(bf16)
        wv = W_qkv[:, 2 * H + cols].astype(bf16)
        bq = np.ascontiguousarray(b_qkv[cols].reshape(2, 128).T)
        wo = np.ascontiguousarray(W_out[h0 * DK : (h0 + HPC) * DK, :])
        in_maps.append(
            {
                "xt": xts[b],
                "wq": np.ascontiguousarray(wq),
                "wk": np.ascontiguousarray(wk),
                "wv": np.ascontiguousarray(wv),
                "wout": wo,
                "bqt": bq,
                "maskbias": mbs[b],
            }
        )
    return in_maps


def _combine(partials, b_qkv, W_out, b_out, perms):
    # b_v contributes b_v @ W_out to every row (softmax weights sum to 1)
    const = (b_out + b_qkv[2 * H :] @ W_out).astype(np.float32)
    out = np.empty((B, T, H), dtype=np.float32)
    for b in range(B):
        acc = partials[4 * b].astype(np.float32)
        for i in range(1, 4):
            acc = acc + partials[4 * b + i]
        out[b][perms[b]] = acc + const[None, :]
    return out


def kernel(x, mask, W_qkv, b_qkv, W_out, b_out):
    x = np.asarray(x, dtype=np.float32)
    mask = np.asarray(mask)
    W_qkv = np.asarray(W_qkv, dtype=np.float32)
    b_qkv = np.asarray(b_qkv, dtype=np.float32)
    W_out = np.asarray(W_out, dtype=np.float32)
    b_out = np.asarray(b_out, dtype=np.float32)

    perms, nvs, nkt = _plan(mask)
    nc = _get_nc(nkt)
    in_maps = _prep_in_maps(x, mask, W_qkv, b_qkv, W_out, perms, nvs, nkt)

    from concourse.bass_utils import run_bass_kernel_spmd

    res = run_bass_kernel_spmd(nc, in_maps, list(range(NCORES)))
    partials = [res.results[c]["out_partial"] for c in range(NCORES)]
    return _combine(partials, b_qkv, W_out, b_out, perms)


# revision 19
# speedup vs baseline: 1.3887x; 1.3887x over previous
"""Multi-head attention (B=2, T=2048, H=1024, 16 heads) on 8 trn2 cores.

Sharding: data-parallel over batch (2) x tensor-parallel over head groups
(4 heads/core).  Each core computes the qkv projection for its 4 heads,
attention, and a partial out-projection; the host sums the 4 partials per
batch and adds the bias constants.

Key optimizations over the straightforward layout:
  * The mask is per-key and known on the host: tokens are permuted so
    valid keys come first, and k/v projections + scores + exp + AV run
    only over ceil(nv/128)*128 keys (~1152 of 2048).  Exact: masked keys
    contribute exp(-1e9)=0.  Queries/outputs use all tokens (computed in
    permuted order, unpermuted on the host).
  * x is transposed on the host (xT [H, T]) -> no PE transposes at all.
  * v is projected directly in key-major layout [tok, feat] using xT
    blocks as the stationary operand -> no V transposes; a ones column
    per key tile gives the softmax denominator for free in the AV matmul.
  * x / W_qkv / ex / vp are bf16 (same PE rate as f32r, half the traffic);
    scores and the out-projection stay f32r.
  * b_k is dropped (softmax shift invariance - exact), b_v is folded into
    a host-side output constant (softmax weights sum to 1 - exact), b_q
    is applied as a per-partition DVE tensor_scalar add on the qT copy.
  * The whole attention phase is one software-pipelined PE stream:
    ss(kt) and av(kt-2) interleave at key-tile granularity so the PE
    tracks the scalar-engine exp pipeline without stalls; projection /
    out-projection units are woven in at block boundaries; the softmax
    denominator reciprocal uses the fast DVE approximation and its PE
    broadcast is deferred into the next block (and lands in the unused
    partitions of the accumulator PSUM tile, saving a bank).
"""

import sys

sys.path.insert(0, "/opt/trn_rl_repo")

import numpy as np

B, T, H = 2, 2048, 1024
NH, DK = 16, 64
HPC = 4            # heads per core
NCORES = 8
NT = T // 128      # 16 token tiles
KTH = H // 128     # 8 hidden contraction tiles

_CACHE = {}


def _build(nkt):
    """Build the per-core kernel for nkt key tiles (KP = nkt*128 keys)."""
    import concourse.bacc as bacc
    import concourse.mybir as mybir
    import concourse.tile as tile

    f32 = mybir.dt.float32
    f32r = mybir.dt.float32r
    bf16 = mybir.dt.bfloat16
    AF = mybir.ActivationFunctionType
    ALU = mybir.AluOpType

    KP = nkt * 128

    nc = bacc.Bacc("TRN2", target_bir_lowering=False, debug=False)

    xt_d = nc.dram_tensor("xt", [H, T], bf16, kind="ExternalInput")
    wq_d = nc.dram_tensor("wq", [H, 256], bf16, kind="ExternalInput")
    wk_d = nc.dram_tensor("wk", [H, 256], bf16, kind="ExternalInput")
    wv_d = nc.dram_tensor("wv", [H, 256], bf16, kind="ExternalInput")
    wout_d = nc.dram_tensor("wout", [256, H], f32r, kind="ExternalInput")
    bq_d = nc.dram_tensor("bqt", [128, 2], f32, kind="ExternalInput")
    maskb_d = nc.dram_tensor("maskbias", [128, nkt], f32, kind="ExternalInput")
    out_d = nc.dram_tensor("out_partial", [T, H], f32, kind="ExternalOutput")

    # k-projection column chunks (cover KP, each >=256 wide for f32r rate)
    if nkt % 3 == 0:
        kchunks = [(i * KP // 3, KP // 3) for i in range(3)]
    else:
        kchunks = [(o, min(512, KP - o)) for o in range(0, KP, 512)]
    nkc = len(kchunks)

    with tile.TileContext(nc) as tc:
        with (
            tc.tile_pool(name="persist", bufs=1) as pp,
            tc.tile_pool(name="small", bufs=1) as sp,
            tc.tile_pool(name="expp", bufs=6) as ep,
            tc.tile_pool(name="recipp", bufs=2) as rp,
            tc.tile_pool(name="ostage", bufs=4) as osp,
            tc.tile_pool(name="psum", bufs=1, space="PSUM") as psp,
        ):
            ones_f = sp.tile([128, 64], f32, tag="ones_f", name="ones_f")
            nc.vector.memset(ones_f, 1.0)
            bqt = sp.tile([128, 2], f32, tag="bqt", name="bqt")
            nc.sync.dma_start(out=bqt, in_=bq_d[:, :])
            maskb = sp.tile([128, nkt], f32, tag="maskb", name="maskb")
            nc.sync.dma_start(out=maskb, in_=maskb_d[:, :])

            # weights (gpsimd queue): small first, wout last
            wq = sp.tile([128, KTH * 256], bf16, tag="wq", name="wq")
            wk = sp.tile([128, KTH * 256], bf16, tag="wk", name="wk")
            wv = sp.tile([128, KTH * 256], bf16, tag="wv", name="wv")
            for w_s, w_d in ((wk, wk_d), (wv, wv_d), (wq, wq_d)):
                for kt in range(KTH):
                    nc.gpsimd.dma_start(
                        out=w_s[:, kt * 256 : (kt + 1) * 256],
                        in_=w_d[kt * 128 : (kt + 1) * 128, :],
                    )
            wout = sp.tile([128, 2 * H], f32r, tag="wout", name="wout")
            for p in range(2):
                nc.gpsimd.dma_start(
                    out=wout[:, p * H : (p + 1) * H],
                    in_=wout_d[p * 128 : (p + 1) * 128, :],
                )

            # xT (sync queue), chunk-major so k/v/q chunk c of every hidden
            # tile lands before chunk c+1 of any.
            xT = pp.tile([128, KTH * T], bf16, tag="xT", name="xT")
            xcols = [(o, w) for (o, w) in kchunks]
            o_end = kchunks[-1][0] + kchunks[-1][1]
            while o_end < T:
                w = min(512, T - o_end)
                xcols.append((o_end, w))
                o_end += w
            for o, w in xcols:
                for kt in range(KTH):
                    nc.sync.dma_start(
                        out=xT[:, kt * T + o : kt * T + o + w],
                        in_=xt_d[kt * 128 : (kt + 1) * 128, o : o + w],
                    )

            # persistent activations
            qT = [
                pp.tile([128, T], f32r, tag=f"qT{m}", name=f"qT{m}")
                for m in range(2)
            ]
            kT = [
                pp.tile([128, KP], f32r, tag=f"kT{m}", name=f"kT{m}")
                for m in range(2)
            ]
            vp = [
                pp.tile([128, nkt * 65], bf16, tag=f"vp{h}", name=f"vp{h}")
                for h in range(HPC)
            ]
            for h in range(HPC):
                # ones column at slot 64 of each 65-wide key tile block
                vpv = vp[h].rearrange("p (t c) -> p t c", c=65)
                nc.scalar.copy(vpv[:, :, 64], ones_f[:, 0:nkt])
            attn = [
                pp.tile([128, T], f32r, tag=f"attnp{p}", name=f"attnp{p}")
                for p in range(2)
            ]

            def qproj_chunk(mt, c):
                # qT[mt][:, c*512:+512]  (psum shares the "ss" ring)
                ps = psp.tile([128, 1024], f32, tag="ss", bufs=2, name="ps")
                ps = ps[:, 0:512]
                for kt in range(KTH):
                    nc.tensor.matmul(
                        ps,
                        wq[:, kt * 256 + mt * 128 : kt * 256 + (mt + 1) * 128],
                        xT[:, kt * T + c * 512 : kt * T + c * 512 + 512],
                        start=(kt == 0),
                        stop=(kt == KTH - 1),
                    )
                nc.vector.tensor_scalar(
                    out=qT[mt][:, c * 512 : c * 512 + 512],
                    in0=ps,
                    scalar1=bqt[:, mt : mt + 1],
                    scalar2=None,
                    op0=ALU.add,
                )

            def kproj_chunk(mt, c):
                o, w = kchunks[c]
                ps = psp.tile([128, 1024], f32, tag="ss", bufs=2, name="ps")
                for kt in range(KTH):
                    nc.tensor.matmul(
                        ps[:, 0:w],
                        wk[:, kt * 256 + mt * 128 : kt * 256 + (mt + 1) * 128],
                        xT[:, kt * T + o : kt * T + o + w],
                        start=(kt == 0),
                        stop=(kt == KTH - 1),
                    )
                nc.vector.tensor_copy(kT[mt][:, o : o + w], ps[:, 0:w])

            def vproj_tile(tt):
                # v for all 4 heads at token tile tt, direct [tok, feat]
                pv = psp.tile([128, 1024], f32, tag="ss", bufs=2, name="pv")
                for kt in range(KTH):
                    nc.tensor.matmul(
                        pv[:, 0:256],
                        xT[:, kt * T + tt * 128 : kt * T + (tt + 1) * 128],
                        wv[:, kt * 256 : (kt + 1) * 256],
                        start=(kt == 0),
                        stop=(kt == KTH - 1),
                    )
                for h in range(HPC):
                    nc.vector.tensor_copy(
                        vp[h][:, tt * 65 : tt * 65 + 64],
                        pv[:, h * 64 : (h + 1) * 64],
                    )

            def ss_unit(hp, kt, nb):
                ss = psp.tile([128, 1024], f32, tag="ss", bufs=2, name="ss")
                for lh in range(2):
                    r0 = lh * 64
                    nc.tensor.matmul(
                        ss[:, lh * 512 : (lh + 1) * 512],
                        kT[hp][r0 : r0 + 64, kt * 128 : (kt + 1) * 128],
                        qT[hp][r0 : r0 + 64, nb * 512 : nb * 512 + 512],
                        start=True,
                        stop=True,
                    )
                ex = ep.tile([128, 1024], bf16, tag="ex", name="ex")
                nc.scalar.activation(
                    ex, ss, AF.Exp, bias=maskb[:, kt : kt + 1], scale=1.0
                )
                return ex

            def av_unit(hp, kt, acc, ex):
                for lh in range(2):
                    h = hp * 2 + lh
                    nc.tensor.matmul(
                        acc[0:65, lh * 512 : (lh + 1) * 512],
                        vp[h][:, kt * 65 : kt * 65 + 65],
                        ex[:, lh * 512 : (lh + 1) * 512],
                        start=(kt == 0),
                        stop=(kt == nkt - 1),
                    )

            def block_recip(acc):
                # fast approximate 1/denominator.  The custom DVE op only
                # reads partition 0, so stage the psum denominator row into
                # a partition-0 SBUF tile first (and keep each call within
                # 512 elements, the HW-validated shape).
                den = rp.tile([1, 1024], f32, tag="den", name="den")
                nc.vector.tensor_copy(den, acc[64:65, :])
                rec = rp.tile([1, 1024], f32, tag="rec", name="rec")
                for lh in range(2):
                    sl = slice(lh * 512, lh * 512 + 512)
                    nc.vector.reciprocal_approx_fast(rec[:, sl], den[:, sl])
                return rec

            def block_finish(hp, nb, acc, rec):
                # PE broadcast of 1/denom into acc rows 64:128, then the
                # normalize multiply into the attn tile.
                for lh in range(2):
                    r0 = lh * 64
                    sl = slice(lh * 512, lh * 512 + 512)
                    nc.tensor.matmul(
                        acc[64:128, sl],
                        ones_f[0:1, :],
                        rec[:, sl],
                        start=True,
                        stop=True,
                    )
                    recb = rp.tile([64, 512], f32, tag="recb", name="recb")
                    nc.vector.tensor_copy(recb, acc[64:128, sl])
                    nc.vector.tensor_tensor(
                        out=attn[hp][r0 : r0 + 64, nb * 512 : nb * 512 + 512],
                        in0=acc[0:64, sl],
                        in1=recb,
                        op=ALU.mult,
                    )

            def outproj_mt(mt):
                for ob in range(2):
                    po = psp.tile([128, 1024], f32, tag="ss", bufs=2, name="po")
                    po = po[:, 0:512]
                    for p in range(2):
                        nc.tensor.matmul(
                            po,
                            attn[p][:, mt * 128 : (mt + 1) * 128],
                            wout[:, p * H + ob * 512 : p * H + ob * 512 + 512],
                            start=(p == 0),
                            stop=(p == 1),
                        )
                    ot = osp.tile([128, 512], f32, tag="ot", name="ot")
                    nc.any.tensor_copy(ot, po)
                    nc.sync.dma_start(
                        out=out_d[
                            mt * 128 : (mt + 1) * 128, ob * 512 : ob * 512 + 512
                        ],
                        in_=ot,
                    )

            # ---- schedule ----
            # Emission order defines dependencies (single in-order queue
            # per engine): everything a unit reads must be emitted first.

            # Phase A: minimum for attention pair 0 block 0.
            for c in range(nkc):
                kproj_chunk(0, c)
            for tt in range(nkt):
                vproj_tile(tt)
            qproj_chunk(0, 0)

            blocks = [(hp, nb) for hp in range(2) for nb in range(4)]
            # entry fillers per block index: consumers are >=1 block later
            entry = {
                0: [lambda: qproj_chunk(0, 1)],
                1: [lambda: qproj_chunk(0, 2), lambda: kproj_chunk(1, 0)],
                2: [
                    lambda: qproj_chunk(0, 3),
                    lambda: kproj_chunk(1, 1),
                    lambda: qproj_chunk(1, 0),
                ],
                3: [lambda c=c: kproj_chunk(1, c) for c in range(2, nkc)]
                + [lambda: qproj_chunk(1, 1), lambda: qproj_chunk(1, 2)],
                4: [lambda: qproj_chunk(1, 3)],
            }
            lag = 2 if nkt > 2 else 1
            pending = None  # (hp, nb, acc, rec) of the previous block
            for bi, (hp, nb) in enumerate(blocks):
                for u in entry.get(bi, []):
                    u()
                acc = psp.tile(
                    [128, 1024], f32, tag="acc", bufs=2, name="acc"
                )
                exs = []
                for kt in range(nkt):
                    exs.append(ss_unit(hp, kt, nb))
                    if kt == 1 and pending is not None:
                        block_finish(*pending[:2], pending[2], pending[3])
                        pending = None
                    if kt >= lag:
                        av_unit(hp, kt - lag, acc, exs[kt - lag])
                    # weave out-projection of pair-1 block nb-1 into this
                    # block once its attn rows exist (finish ran at kt==1)
                    if hp == 1 and nb > 0 and kt in (4, 7):
                        base = (nb - 1) * 4 + (0 if kt == 4 else 2)
                        outproj_mt(base)
                        outproj_mt(base + 1)
                for kt in range(nkt - lag, nkt):
                    av_unit(hp, kt, acc, exs[kt])
                rec = block_recip(acc)
                pending = (hp, nb, acc, rec)

            # tail: finish last block, then its out-projection
            block_finish(*pending[:2], pending[2], pending[3])
            for mt in range(12, 16):
                outproj_mt(mt)

    nc.compile()
    return nc


def _get_nc(nkt):
    key = f"nc{nkt}"
    if key not in _CACHE:
        _CACHE[key] = _build(nkt)
    return _CACHE[key]


def _plan(mask):
    """Per-batch token permutation (valid keys first) and key-tile count."""
    m = np.asarray(mask)[:, 0, 0, :]
    perms, nvs = [], []
    for b in range(B):
        valid = np.nonzero(m[b] != 0)[0]
        invalid = np.nonzero(m[b] == 0)[0]
        perms.append(np.concatenate([valid, invalid]))
        nvs.append(len(valid))
    nkt = max(1, -(-max(nvs) // 128))  # ceil; same program on all cores
    nkt = min(nkt, NT)
    return perms, nvs, nkt


def _prep_in_maps(x, mask, W_qkv, b_qkv, W_out, perms, nvs, nkt):
    import ml_dtypes

    bf16 = ml_dtypes.bfloat16
    KP = nkt * 128
    xts = []
    mbs = []
    for b in range(B):
        xts.append(np.ascontiguousarray(x[b][perms[b]].T.astype(bf16)))
        mb = np.zeros(KP, dtype=np.float32)
        mb[nvs[b] :] = -1e9
        mbs.append(np.ascontiguousarray(mb.reshape(nkt, 128).T))

    in_maps = []
    for c in range(NCORES):
        b = c // 4
        h0 = (c % 4) * HPC
        cols = np.arange(h0 * DK, (h0 + HPC) * DK)
        wq = W_qkv[:, cols].astype(bf16)
        wk = (W_qkv[:, H + cols] * 0.125).astype(bf16)
        wv = W_qkv[:, 2 * H + cols].astype(bf16)
        bq = np.ascontiguousarray(b_qkv[cols].reshape(2, 128).T)
        wo = np.ascontiguousarray(W_out[h0 * DK : (h0 + HPC) * DK, :])
        in_maps.append(
            {
                "xt": xts[b],
                "wq": np.ascontiguousarray(wq),
                "wk": np.ascontiguousarray(wk),
                "wv": np.ascontiguousarray(wv),
                "wout": wo,
                "bqt": bq,
                "maskbias": mbs[b],
            }
        )
    return in_maps


def _combine(partials, b_qkv, W_out, b_out, perms):
    # b_v contributes b_v @ W_out to every row (softmax weights sum to 1)
    const = (b_out + b_qkv[2 * H :] @ W_out).astype(np.float32)
    out = np.empty((B, T, H), dtype=np.float32)
    for b in range(B):
        acc = partials[4 * b].astype(np.float32)
        for i in range(1, 4):
            acc = acc + partials[4 * b + i]
        out[b][perms[b]] = acc + const[None, :]
    return out


def kernel(x, mask, W_qkv, b_qkv, W_out, b_out):
    x = np.asarray(x, dtype=np.float32)
    mask = np.asarray(mask)
    W_qkv = np.asarray(W_qkv, dtype=np.float32)
    b_qkv = np.asarray(b_qkv, dtype=np.float32)
    W_out = np.asarray(W_out, dtype=np.float32)
    b_out = np.asarray(b_out, dtype=np.float32)

    perms, nvs, nkt = _plan(mask)
    nc = _get_nc(nkt)
    in_maps = _prep_in_maps(x, mask, W_qkv, b_qkv, W_out, perms, nvs, nkt)

    from concourse.bass_utils import run_bass_kernel_spmd

    res = run_bass_kernel_spmd(nc, in_maps, list(range(NCORES)))
    partials = [res.results[c]["out_partial"] for c in range(NCORES)]
    return _combine(partials, b_qkv, W_out, b_out, perms)
